# revision 12
# baseline (speedup 1.0000x reference)
"""GroupedQueryAttentionWithSink — Trainium2 Bass/Tile kernel, 8 NeuronCores.

Sharding: tensor-parallel over heads. Each core owns 4 Q heads + its 1 KV head
(heads 4c..4c+3 / kv head c), computes the full causal attention for those
heads plus its 256-row slice of the output projection; host sums the 8
partial outputs (out = sum_c A_c @ Wo_c).

Device-side layout is "scores transposed" flash attention:
  Q^T [64, S] per head, K^T duplicated [128, S], V natural [S, 64] (+ ones col)
  scoresT[k, q] = K^T.T-chunk @ Q^T     (PE, fp32r)
  expS = exp(scoresT * ck/8)            (ACT, per-partition k-norm scale)
  out^T[65, q] += Vaug.T @ expS         (PE, row 64 accumulates softmax denom)
RMSNorm is folded in as per-vector scalars (c_q pre-scales Q^T columns,
c_k folds into the exp scale); RoPE is applied with host-prepared
cos/sin tables (norm weights folded) using DMA partition-swapped copies.
"""

import os
import numpy as np

import concourse.bass as bass
import concourse.mybir as mybir
import concourse.tile as tile
from concourse import bacc
from concourse.bass_utils import run_bass_kernel_spmd

S = 2048
D = 2048
HD = 64
NHEADS = 32
NKV = 8
NCORES = 8
NH_LOC = NHEADS // NCORES      # 4 q heads per core
SB = 512                       # seq block (psum free dim)
NSB = S // SB                  # 4
NDC = D // 128                 # 16 contraction chunks
NSC = S // 128                 # 16 seq chunks of 128
EPS = 1e-6

f32 = mybir.dt.float32
f32r = mybir.dt.float32r

# test.py can flip this to get a profiled run
TRACE = False
TRACE_DIR = None
LAST_EXEC_NS = None
LAST_RESULTS = None

_CACHE = {}
DEBUG_DUMPS = False


def _build_bass():
    nc = bacc.Bacc("TRN2", debug=False, num_devices=NCORES)

    # ---- DRAM parameters (identical program on all 8 cores) ----
    xT = nc.dram_tensor("xT", [D, S], f32r, kind="ExternalInput").ap()
    wq = nc.dram_tensor("wq", [D, NH_LOC * HD], f32r, kind="ExternalInput").ap()
    wkv = nc.dram_tensor("wkv", [D, 2 * HD], f32r, kind="ExternalInput").ap()
    wo = nc.dram_tensor("wo", [NH_LOC * HD, D], f32r, kind="ExternalInput").ap()
    cosq_d = nc.dram_tensor("cosq", [128, S], f32, kind="ExternalInput").ap()
    sinq_d = nc.dram_tensor("sinq", [128, S], f32, kind="ExternalInput").ap()
    cosk_d = nc.dram_tensor("cosk", [128, S], f32, kind="ExternalInput").ap()
    sink_d = nc.dram_tensor("sink", [128, S], f32, kind="ExternalInput").ap()
    maskt_d = nc.dram_tensor("maskt", [NSB, 128, SB], f32, kind="ExternalInput").ap()
    sinkexp_d = nc.dram_tensor("sinkexp", [128, NH_LOC], f32, kind="ExternalInput").ap()
    ident_d = nc.dram_tensor("ident", [HD, HD], f32, kind="ExternalInput").ap()
    outT = nc.dram_tensor("outT", [D, S], f32, kind="ExternalOutput").ap()
    if DEBUG_DUMPS:
        qt_dbg = nc.dram_tensor("qt_dbg", [128, 2, S], f32, kind="ExternalOutput").ap()
        kdup_dbg = nc.dram_tensor("kdup_dbg", [128, S], f32, kind="ExternalOutput").ap()
        ck8_dbg = nc.dram_tensor("ck8_dbg", [128, 2 * NSC], f32, kind="ExternalOutput").ap()
        vaug_dbg = nc.dram_tensor("vaug_dbg", [128, NSC, HD + 1], f32, kind="ExternalOutput").ap()
        atc_dbg = nc.dram_tensor("atc_dbg", [NSB, 128, 2, SB], f32, kind="ExternalOutput").ap()

    mult = mybir.AluOpType.mult
    add = mybir.AluOpType.add
    AF = mybir.ActivationFunctionType

    with tile.TileContext(nc) as tc:
        with (
            tc.tile_pool(name="persist", bufs=1) as pp,
            tc.tile_pool(name="work", bufs=2) as wp,
            tc.tile_pool(name="xin", bufs=3) as xp,
            tc.tile_pool(name="es_pool", bufs=3) as esp,
            tc.tile_pool(name="psum", bufs=1, space="PSUM") as psp,
        ):
            # ---- persistent tiles ----
            wq_sb = pp.tile([128, NDC, NH_LOC * HD], f32r, name="wq_sb")
            wkv_sb = pp.tile([128, NDC, 2 * HD], f32r, name="wkv_sb")
            wo_sb = pp.tile([128, 2, D], f32r, name="wo_sb")
            maskt = pp.tile([128, NSB, SB], f32, name="maskt_t")
            sinkexp = pp.tile([128, NH_LOC], f32, name="sinkexp_t")
            ident = pp.tile([HD, HD], f32, name="ident_t")
            ones128 = pp.tile([128, 1], f32r, name="ones128")
            ones2 = pp.tile([128, 2], f32r, name="ones2")
            epsk = pp.tile([128, 1], f32, name="epsk")
            epsq = pp.tile([128, 1], f32, name="epsq")

            qt = pp.tile([128, 2, S], f32r, name="qt")        # Q^T, 2 m-chunks
            kdup = pp.tile([128, S], f32r, name="kdup")       # K^T duplicated
            vaug = pp.tile([128, NSC, HD + 1], f32r, name="vaug")
            ck8 = pp.tile([128, 2 * NSC], f32, name="ck8")       # c_k/8 per k pos

            # ---- constant loads ----
            nc.sync.dma_start(out=wq_sb, in_=wq.rearrange("(dc p) m -> p dc m", p=128))
            nc.sync.dma_start(out=wkv_sb, in_=wkv.rearrange("(dc p) m -> p dc m", p=128))
            nc.sync.dma_start(out=wo_sb, in_=wo.rearrange("(cc p) d -> p cc d", p=128))
            nc.sync.dma_start(out=maskt, in_=maskt_d.rearrange("j p q -> p j q"))
            nc.sync.dma_start(out=sinkexp, in_=sinkexp_d)
            nc.sync.dma_start(out=ident, in_=ident_d)
            nc.vector.memset(ones128.bitcast(f32), 1.0)
            nc.vector.memset(ones2.bitcast(f32), 1.0)
            nc.vector.memset(epsk, 64.0 * EPS)
            nc.vector.memset(epsq, EPS)
            nc.vector.memset(vaug[:, :, HD : HD + 1].bitcast(f32), 1.0)

            for sb in range(NSB):
                ss = slice(sb * SB, (sb + 1) * SB)

                # ---- projections: accumulate over d-chunks ----
                psq0 = psp.tile([128, SB], f32, tag="pj3", name="psq0", bufs=3)
                psq1 = psp.tile([128, SB], f32, tag="pj3", name="psq1", bufs=3)
                pskv = psp.tile([128, SB], f32, tag="pj3", name="pskv", bufs=3)
                for dc in range(NDC):
                    xt = xp.tile([128, SB], f32r, tag="xt", name="xt")
                    nc.sync.dma_start(
                        out=xt, in_=xT[dc * 128 : (dc + 1) * 128, ss]
                    )
                    st = dict(start=(dc == 0), stop=(dc == NDC - 1))
                    nc.tensor.matmul(psq0, (wq_sb[:, dc, 0:128]), (xt), **st)
                    nc.tensor.matmul(psq1, (wq_sb[:, dc, 128:256]), (xt), **st)
                    nc.tensor.matmul(pskv, (wkv_sb[:, dc, :]), (xt), **st)

                # rope tables for this block
                cosq_s = wp.tile([128, SB], f32, tag="cosq_s", name="cosq_s")
                sinq_s = wp.tile([128, SB], f32, tag="sinq_s", name="sinq_s")
                cosk_s = wp.tile([128, SB], f32, tag="cosk_s", name="cosk_s")
                sink_s = wp.tile([128, SB], f32, tag="sink_s", name="sink_s")
                nc.sync.dma_start(out=cosq_s, in_=cosq_d[:, ss])
                nc.sync.dma_start(out=sinq_s, in_=sinq_d[:, ss])
                nc.sync.dma_start(out=cosk_s, in_=cosk_d[:, ss])
                nc.sync.dma_start(out=sink_s, in_=sink_d[:, ss])

                # ---- evacuate + rope ----
                q0raw = wp.tile([128, SB], f32, tag="q0raw", name="q0raw", bufs=1)
                q1raw = wp.tile([128, SB], f32, tag="q1raw", name="q1raw", bufs=1)
                kvraw = wp.tile([128, SB], f32, tag="kvraw", name="kvraw", bufs=1)
                nc.any.tensor_copy(q0raw, psq0)
                nc.any.tensor_copy(q1raw, psq1)
                nc.any.tensor_copy(kvraw, pskv)

                # partition-swapped copies via DMA (rope rotate-half)
                q0sw = wp.tile([128, SB], f32, tag="q0sw", name="q0sw", bufs=1)
                q1sw = wp.tile([128, SB], f32, tag="q1sw", name="q1sw", bufs=1)
                ksw = wp.tile([128, SB], f32, tag="ksw", name="ksw", bufs=1)
                kdr = wp.tile([128, SB], f32, tag="kdr", name="kdr", bufs=1)
                for qraw, qsw in ((q0raw, q0sw), (q1raw, q1sw)):
                    for g in range(4):
                        src = (g ^ 1) * 32
                        nc.sync.dma_start(
                            out=qsw[g * 32 : (g + 1) * 32, :],
                            in_=qraw[src : src + 32, :],
                        )
                # K: duplicate rows 0-63 into both halves, plus swapped copy
                for half in range(2):
                    nc.sync.dma_start(
                        out=kdr[half * 64 : half * 64 + 64, :], in_=kvraw[0:64, :]
                    )
                    for g in range(2):
                        src = (g ^ 1) * 32
                        nc.sync.dma_start(
                            out=ksw[half * 64 + g * 32 : half * 64 + (g + 1) * 32, :],
                            in_=kvraw[src : src + 32, :],
                        )
                # V^T slice (partition move 64..128 -> 0..64)
                vtp_sb = wp.tile([HD, SB], f32, tag="vtp_sb", name="vtp_sb")
                nc.sync.dma_start(out=vtp_sb, in_=kvraw[64:128, :])

                # rope: out = raw*cos + swapped*sin  (tables have norm-w folded)
                for raw, sw, cost, sint, dst in (
                    (q0raw, q0sw, cosq_s, sinq_s, qt[:, 0, ss]),
                    (q1raw, q1sw, cosq_s, sinq_s, qt[:, 1, ss]),
                    (kdr, ksw, cosk_s, sink_s, kdup[:, ss]),
                ):
                    rta = wp.tile([128, SB], f32, tag="rt", name="rta", bufs=4)
                    rtb = wp.tile([128, SB], f32, tag="rt", name="rtb", bufs=4)
                    nc.vector.tensor_tensor(out=rta, in0=raw, in1=cost, op=mult)
                    nc.vector.tensor_tensor(out=rtb, in0=sw, in1=sint, op=mult)
                    nc.vector.tensor_tensor(out=dst, in0=rta, in1=rtb, op=add)

                # ---- V transpose into Vaug ----
                for j in range(4):
                    sc = 4 * sb + j
                    ptr = psp.tile([128, HD], f32, tag="sm", name="ptr", bufs=2)
                    nc.tensor.transpose(
                        ptr, vtp_sb[:, j * 128 : (j + 1) * 128], ident
                    )
                    nc.any.tensor_copy(vaug[:, sc, 0:HD], ptr)

                # ---- k-norm scale: ck8 = 1/sqrt(sum_d k^2 + 64 eps) = c_k/8
                skt_sb = wp.tile([HD, SB], f32r, tag="skt_sb", name="skt_sb")
                nc.vector.tensor_tensor(
                    out=skt_sb, in0=kdup[0:64, ss], in1=kdup[0:64, ss], op=mult
                )
                psck = psp.tile([128, 8], f32, tag="sm", name="psck", bufs=2)
                for j in range(4):
                    nc.tensor.matmul(
                        psck[:, 2 * j : 2 * j + 2],
                        skt_sb[:, j * 128 : (j + 1) * 128],
                        ones2[0:64, :],
                        start=True,
                        stop=True,
                    )
                cks = wp.tile([128, 8], f32, tag="cks", name="cks")
                nc.scalar.activation(cks, psck, AF.Sqrt, bias=epsk, scale=1.0)
                nc.vector.reciprocal(ck8[:, 8 * sb : 8 * sb + 8], cks)

                # ---- q-norm: cq = 1/sqrt(sum/64 + eps); qt *= bcast(cq) ----
                for mc in range(2):
                    sq = wp.tile([128, SB], f32r, tag="sq", name="sq")
                    nc.vector.tensor_tensor(
                        out=sq, in0=qt[:, mc, ss], in1=qt[:, mc, ss], op=mult
                    )
                    for hh in range(2):
                        base = hh * 64
                        psa = psp.tile([1, SB], f32, tag="sm", name="psa", bufs=2)
                        nc.tensor.matmul(
                            psa,
                            ones128[base : base + 64, :],
                            sq[base : base + 64, :],
                            start=True,
                            stop=True,
                            tile_position=(base, 0),
                        )
                        cqs = wp.tile([1, SB], f32, tag="cqs", name="cqs")
                        nc.scalar.activation(
                            cqs, psa, AF.Sqrt, bias=epsq[0:1, :], scale=1.0 / 64.0
                        )
                        cqr = wp.tile([1, SB], f32, tag="cqr", name="cqr")
                        nc.vector.reciprocal(cqr, cqs)
                        bcq = wp.tile([128, SB], f32, tag="bcq", name="bcq")
                        nc.gpsimd.partition_broadcast(bcq, cqr, channels=128)
                        nc.vector.tensor_tensor(
                            out=qt[base : base + 64, mc, ss],
                            in0=qt[base : base + 64, mc, ss],
                            in1=bcq[base : base + 64, :],
                            op=mult,
                        )

                # ---- attention for q-block qb == sb (all local heads) ----
                qb = sb
                nkc = (qb + 1) * 4
                atc_sb = wp.tile([128, 2, SB], f32r, tag="atc_sb", name="atc_sb")
                for h in range(NH_LOC):
                    base = (h % 2) * 64
                    mc = h // 2
                    pso = psp.tile([HD + 1, SB], f32, tag="po", name="pso", bufs=2)
                    for kc in range(nkc):
                        pss = psp.tile([128, SB], f32, tag="pj3", name="pss", bufs=3)
                        nc.tensor.matmul(
                            pss,
                            (kdup[base : base + 64, kc * 128 : (kc + 1) * 128]),
                            (qt[base : base + 64, mc, qb * SB : (qb + 1) * SB]),
                            start=True,
                            stop=True,
                            tile_position=(base, 0),
                        )
                        es = esp.tile([128, SB], f32r, tag="es", name="es")
                        nc.scalar.activation(
                            es, pss, AF.Exp, scale=ck8[:, 2 * kc : 2 * kc + 1]
                        )
                        j = kc - qb * 4
                        if j >= 0:
                            nc.vector.tensor_tensor(
                                out=es, in0=es, in1=maskt[:, j, :], op=mult
                            )
                        nc.tensor.matmul(
                            pso,
                            (vaug[:, kc, :]),
                            (es),
                            start=(kc == 0),
                            stop=(kc == nkc - 1),
                        )
                    # epilogue: denom = row64 + exp(sink); at = pso[0:64]/denom
                    den = wp.tile([HD + 1, SB], f32, tag="den", name="den", bufs=1)
                    nc.vector.tensor_scalar_add(
                        den[64:65, :], pso[64:65, :], sinkexp[64:65, h : h + 1]
                    )
                    rec = wp.tile([HD + 1, SB], f32, tag="rec", name="rec", bufs=1)
                    nc.vector.reciprocal(rec[64:65, :], den[64:65, :])
                    rec0 = wp.tile([1, SB], f32, tag="rec0", name="rec0")
                    nc.sync.dma_start(out=rec0, in_=rec[64:65, :])
                    bcd = wp.tile([64, SB], f32, tag="bcd", name="bcd")
                    nc.gpsimd.partition_broadcast(bcd, rec0, channels=64)
                    ato = wp.tile([64, SB], f32r, tag="ato", name="ato")
                    nc.vector.tensor_tensor(out=ato, in0=pso[0:64, :], in1=bcd, op=mult)
                    nc.sync.dma_start(
                        out=atc_sb[(h % 2) * 64 : (h % 2) * 64 + 64, h // 2, :],
                        in_=ato,
                    )

                if DEBUG_DUMPS:
                    nc.sync.dma_start(out=atc_dbg[sb], in_=atc_sb.bitcast(f32))

                # ---- output projection for s-block sb ----
                for dch in range(NDC):
                    po = psp.tile([128, SB], f32, tag="po", name="po", bufs=2)
                    for cc in range(2):
                        nc.tensor.matmul(
                            po,
                            (wo_sb[:, cc, dch * 128 : (dch + 1) * 128]),
                            (atc_sb[:, cc, :]),
                            start=(cc == 0),
                            stop=(cc == 1),
                        )
                    ot = wp.tile([128, SB], f32, tag="ot", name="ot", bufs=3)
                    nc.any.tensor_copy(ot, po)
                    nc.sync.dma_start(
                        out=outT[dch * 128 : (dch + 1) * 128, ss], in_=ot
                    )

            if DEBUG_DUMPS:
                nc.sync.dma_start(out=qt_dbg, in_=qt.bitcast(f32))
                nc.sync.dma_start(out=kdup_dbg, in_=kdup.bitcast(f32))
                nc.sync.dma_start(out=ck8_dbg, in_=ck8)
                nc.sync.dma_start(out=vaug_dbg, in_=vaug.bitcast(f32))

    nc.compile()
    return nc


def _get_nc():
    if "nc" not in _CACHE:
        _CACHE["nc"] = _build_bass()
    return _CACHE["nc"]


def _host_prep(x, mask, cos, sin, wq, wk, wv, wo, q_norm_w, k_norm_w, sink):
    """Build the 8 per-core input maps."""
    f = np.float32
    x = np.asarray(x, f)
    cos = np.asarray(cos, f)
    sin = np.asarray(sin, f)
    wq = np.asarray(wq, f)
    wk = np.asarray(wk, f)
    wv = np.asarray(wv, f)
    wo = np.asarray(wo, f)
    qw = np.asarray(q_norm_w, f)
    kw = np.asarray(k_norm_w, f)
    sink_l = np.asarray(sink, f)
    mask = np.asarray(mask)

    xT = np.ascontiguousarray(x[0].T)  # [D, S]

    d = np.arange(HD)
    swap = np.where(d < 32, d + 32, d - 32)
    sign = np.where(d < 32, -1.0, 1.0).astype(f)

    def rope_tables(w):
        c = (cos.T * w[:, None]).astype(f)                      # [64, S]
        s = (sin.T * (sign * w[swap])[:, None]).astype(f)       # [64, S]
        return (
            np.ascontiguousarray(np.vstack([c, c])),
            np.ascontiguousarray(np.vstack([s, s])),
        )

    cosq_t, sinq_t = rope_tables(qw)
    cosk_t, sink_t = rope_tables(kw)

    # causal 0/1 mask patterns for the 4 diagonal 128-chunks of a 512 q-block
    maskt = np.empty((NSB, 128, SB), f)
    for j in range(NSB):
        maskt[j] = (~mask[0:SB, j * 128 : (j + 1) * 128]).T.astype(f)

    ident = np.eye(HD, dtype=f)

    in_maps = []
    for c in range(NCORES):
        hs = slice(c * NH_LOC * HD, (c + 1) * NH_LOC * HD)
        kv = slice(c * HD, (c + 1) * HD)
        sinkexp = np.broadcast_to(
            np.exp(sink_l[c * NH_LOC : (c + 1) * NH_LOC]).astype(f), (128, NH_LOC)
        ).copy()
        in_maps.append(
            {
                "xT": xT,
                "wq": np.ascontiguousarray(wq[:, hs]),
                "wkv": np.ascontiguousarray(
                    np.concatenate([wk[:, kv], wv[:, kv]], axis=1)
                ),
                "wo": np.ascontiguousarray(wo[hs, :]),
                "cosq": cosq_t,
                "sinq": sinq_t,
                "cosk": cosk_t,
                "sink": sink_t,
                "maskt": maskt,
                "sinkexp": sinkexp,
                "ident": ident,
            }
        )
    return in_maps


def kernel(**inputs):
    global LAST_EXEC_NS, LAST_RESULTS
    in_maps = _host_prep(**inputs)
    nc = _get_nc()
    res = run_bass_kernel_spmd(
        nc, in_maps, list(range(NCORES)), trace=TRACE, tmpdir=TRACE_DIR
    )
    LAST_EXEC_NS = res.exec_time_ns
    LAST_RESULTS = res
    parts = np.stack([r["outT"] for r in res.results])  # [8, D, S]
    outT = parts.sum(axis=0, dtype=np.float32)
    return np.ascontiguousarray(outT.T).reshape(1, S, D).astype(np.float32)
